# revision 30
# baseline (speedup 1.0000x reference)
"""Trainium2 Bass kernel for the constrained Hamiltonian NN (CHNN) vector field.

Math: the reference solves, per batch sample,
    out = JdH - J DPhi X,   A X = DPhi^T JdH,   A = DPhi^T J DPhi  (64x64)
For chain constraints DPhi = [[Dr, Ddot], [0, Minv Dr]] and
    A = [[0, G], [-G, K]],  G = Dr^T Minv Dr  (32x32 tridiagonal SPD),
    K = Ddot^T Minv Dr - Dr^T Minv Ddot      (tridiagonal antisymmetric)
so X = [x0; x1] with x1 = G^{-1} b0 and x0 = G^{-1}(K x1 - b1), where
    b0 = Dr^T v,  b1 = Ddot^T v - Dr^T Minv g,  v = Minv p,  g = dV/dr.
Output: out_r = v - Minv Dr x1 ; out_p = -g + Dr x0 + Ddot x1.

The MLP grad g runs feature-major on the tensor engine in fp32r (batch on
the free axis, batch=512); the tridiagonal solves run batch-major on the
vector engine via parallel cyclic reduction (PCR, 5 levels) over a fused
[128, 16+4*32+16] layout: the four per-core 32-constraint systems sit
contiguously, and the zero couplings at chunk boundaries make shifted
reads across chunks harmless, so every PCR op is one flat 2D instruction.
Per-level coefficients are kept so the second solve is a rhs-only replay.
softplus/sigmoid use Exp/Ln only => a single activation table load; the
backward pass is carried negated so sigma = 1 - exp(-h) needs no extra op.

Sharding: pure data-parallel over the batch axis across 8 cores.
"""

import numpy as np

N, D = 32, 2
ZD = 128            # state dim
BS = 4096           # full batch
NCORES = 8
BSL = BS // NCORES  # 512 per core
C4 = BSL // 128     # 4 batch chunks of 128 partitions
PAD = 16            # PCR pad (max shift); active cols PAD..PAD+128

# packed fp32r weight block column offsets (per partition p):
#   W0ext (256; row 64 = b0) | W1 2x256 | W2 2x256 | W2T(w3-folded) 2x256 |
#   W1T 2x256 | W0T 2x64 | bias rows b1|b2 (512, row 0 only)
OW0, OW1, OW2, OW2T, OW1T, OW0T, OBR, OON, WTOT = 0, 256, 768, 1280, 1792, 2304, 2432, 2944, 3456
# packed fp32 block: b0(2) b1(2) b2(2) w3(2) invm(4x64) jnv4(4x32) fco(4x32) eco(4x32)
OB0, OB1, OB2, OW3, OIV, OJN, OFC, OEC, CTOT = 0, 2, 4, 6, 8, 264, 392, 520, 648


def build_program(debug=False):
    """Build + compile the single-core SPMD Bass/Tile program."""
    from contextlib import ExitStack

    import concourse.bass as bass
    import concourse.mybir as mybir
    import concourse.tile as tile
    from concourse import bacc
    from concourse.hw_specs import get_activation_tables
    from concourse.masks import make_identity
    import bass_rust as _bass_rust

    f32 = mybir.dt.float32
    f32r = mybir.dt.float32r
    AF = mybir.ActivationFunctionType
    OP = mybir.AluOpType

    class PinnedActBacc(bacc.Bacc):
        # Keep every ACT op on one table (Exp+Ln live together in
        # natural_log_exp_and_others); emptying the others preserves the
        # act_func_set_id indexing while forcing a single table load.
        def insert_act_table_loads(self):
            has_activation = any(
                isinstance(i, mybir.InstActivation)
                for b in self.main_func.blocks
                for i in b.instructions
            )
            if not has_activation:
                return
            tables = [
                (name, funcs if name == "natural_log_exp_and_others" else set())
                for name, funcs in get_activation_tables(self.m.arch).items()
            ]
            _bass_rust.insert_act_table_loads(self, tables)

    nc = PinnedActBacc(
        "TRN2",
        target_bir_lowering=False,
        debug=debug,
        enable_asserts=True,
        num_devices=NCORES,
    )

    z = nc.dram_tensor("z", [BSL, ZD], f32, kind="ExternalInput")
    wpk = nc.dram_tensor("wpk", [128, WTOT], f32r, kind="ExternalInput")
    cpk = nc.dram_tensor("cpk", [128, CTOT], f32, kind="ExternalInput")
    out = nc.dram_tensor("out", [BSL, ZD], f32, kind="ExternalOutput")

    with tile.TileContext(nc) as tc:
        with ExitStack() as ctx:
            const = ctx.enter_context(tc.tile_pool(name="const", bufs=1))
            main = ctx.enter_context(tc.tile_pool(name="main", bufs=1))
            scr = ctx.enter_context(tc.tile_pool(name="scr", bufs=2))
            psmm = ctx.enter_context(tc.tile_pool(name="psmm", bufs=3, space="PSUM"))
            pstr = ctx.enter_context(tc.tile_pool(name="pstr", bufs=2, space="PSUM"))

            # ---- input DMAs: z first; consts split over parallel queues ----
            # 4 consecutive batch rows per partition -> 2KB DMA lines; the
            # within-core batch permutation (sample 4p+j at [p, j]) is
            # consistent across rT/g transposes and the output writeback.
            zt = main.tile([128, C4, ZD], f32)
            nc.sync.dma_start(out=zt, in_=z.ap().rearrange("(p j) f -> p j f", j=4))
            csb = const.tile([128, CTOT], f32)
            nc.sync.dma_start(out=csb, in_=cpk.ap())
            # weight groups as separate tiles (per-group DMA deps), DMA'd
            # in use order across the two hwdge queues
            wg0 = const.tile([128, 256], f32r)   # W0
            wg1 = const.tile([128, 512], f32r)   # W1
            wg2 = const.tile([128, 512], f32r)   # W2
            wg3 = const.tile([128, 1152], f32r)  # W2T | W1T | W0T
            nc.scalar.dma_start(out=wg0, in_=wpk.ap()[:, OW0 : OW0 + 256])
            nc.sync.dma_start(out=wg1, in_=wpk.ap()[:, OW1 : OW1 + 512])
            nc.scalar.dma_start(out=wg2, in_=wpk.ap()[:, OW2 : OW2 + 512])
            nc.sync.dma_start(out=wg3, in_=wpk.ap()[:, OW2T : OW2T + 1152])
            wgb = const.tile([1, 512], f32r)   # [b1 | b2] as K=1 weight rows
            nc.scalar.dma_start(out=wgb, in_=wpk.ap()[0:1, OBR : OBR + 512])

            w0sb = wg0[0:65, :]
            w1sb = wg1.rearrange("p (k n) -> p k n", k=2)
            w2sb = wg2.rearrange("p (k n) -> p k n", k=2)
            w2tsb = wg3[:, 0:512].rearrange("p (k n) -> p k n", k=2)
            w1tsb = wg3[:, 512:1024].rearrange("p (k n) -> p k n", k=2)
            w0tsb = wg3[:, 1024:1152].rearrange("p (k n) -> p k n", k=2)
            invmP = csb[:, OIV : OIV + 256]            # [128, 256] flat
            invmP3 = invmP.rearrange("p (c f) -> p c f", f=64)
            jnvP = csb[:, OJN : OJN + 128]             # [128, 128] flat
            fcoP = csb[:, OFC : OFC + 128]
            ecoP = csb[:, OEC : OEC + 128]
            ident = const.tile([128, 128], f32)
            make_identity(nc, ident)

            zt3 = zt  # [128, C4, 128]

            # ================= MLP forward start (PE/ACT heavy) ============
            # biases ride inside the matmuls (ones-row for L0, K=1 bias
            # matmuls for L1/L2), so both m-halves share one [128, 2*BSL]
            # ACT op per stage.
            rT = main.tile([65, BSL], f32r)
            nc.gpsimd.dma_start(out=rT[64:65, :], in_=wpk.ap()[0:1, OON : OON + BSL])
            for c in range(C4):
                pt = pstr.tile([64, 128], f32, tag="pt", name="pt")
                nc.tensor.transpose(pt, zt3[:, c, 0:64], ident)
                nc.scalar.copy(rT[0:64, c * 128 : (c + 1) * 128], pt)
            ones = const.tile([1, BSL], f32r)
            nc.gpsimd.dma_start(out=ones, in_=wpk.ap()[0:1, OON : OON + BSL])

            # softplus h = Ln(Exp(x)+1); then e = Exp(-h): sigma = 1 - e.
            # e0/e1 are emitted late (backward-only) to keep the ACT queue
            # clear for the forward-critical Exp/Ln chain.
            def act_h(ps2, h_dst):
                tE = scr.tile([128, 2, BSL], f32, tag="tE", name="tE")
                nc.scalar.activation(tE, ps2, AF.Exp)
                nc.scalar.activation(h_dst, tE, AF.Ln, bias=1.0)

            def act_e(h_src, e_dst):
                nc.scalar.activation(e_dst, h_src, AF.Exp, scale=-1.0)

            h0 = main.tile([128, 2, BSL], f32r)
            e0 = main.tile([128, 2, BSL], f32)
            ps0 = psmm.tile([128, 2, BSL], f32, tag="mm", name="ps0")
            for m in range(2):
                nc.tensor.matmul(
                    ps0[:, m, :], w0sb[:, m * 128 : (m + 1) * 128], rT,
                    start=True, stop=True,
                )
            act_h(ps0, h0)

            # ---- chain quantities (batch-major); flat [128, 256] tiles ----
            def c3(t):  # [p, C4, 64] view of a flat [p, 256] tile
                return t.rearrange("p (c f) -> p c f", f=64)

            vt = main.tile([128, 256], f32)          # v = Minv p
            nc.gpsimd.tensor_mul(c3(vt), zt3[:, :, 64:128], invmP3)
            ut = main.tile([128, 256], f32)          # u~ = diff(r) (u = 2 u~)
            nc.vector.tensor_sub(
                c3(ut)[:, :, 2:64], zt3[:, :, 0:62], zt3[:, :, 2:64]
            )
            nc.gpsimd.tensor_copy(c3(ut)[:, :, 0:2], zt3[:, :, 0:2])
            wt = main.tile([128, 256], f32)          # w~ = diff(v)
            nc.vector.tensor_sub(
                c3(wt)[:, :, 2:64], c3(vt)[:, :, 0:62], c3(vt)[:, :, 2:64]
            )
            nc.gpsimd.tensor_copy(c3(wt)[:, :, 0:2], c3(vt)[:, :, 0:2])

            # products + pairwise-D sums -> flat [128, 128] per-constraint
            def prodpair(dst, xa, ya, npair_part, mul_eng=None, add_eng=None):
                pr = scr.tile([128, 256], f32, tag="prod", name="pr")
                if npair_part == 32:
                    (mul_eng or nc.vector).tensor_mul(pr, xa, ya)
                else:
                    (mul_eng or nc.vector).tensor_mul(
                        c3(pr)[:, :, 0:62], xa, ya
                    )
                pe = pr.rearrange("p (c i d) -> p c i d", d=2, i=32)
                d3 = dst.rearrange("p (c i) -> p c i", i=32)
                (add_eng or nc.vector).tensor_add(
                    d3[:, :, 0:npair_part],
                    pe[:, :, 0:npair_part, 0],
                    pe[:, :, 0:npair_part, 1],
                )

            dcc = main.tile([128, 128], f32)
            prodpair(dcc, ut, ut, 32)
            pww = main.tile([128, 128], f32)
            prodpair(pww, wt, wt, 32, nc.gpsimd, nc.gpsimd)
            cuu = main.tile([128, 128], f32)
            nc.gpsimd.memset(cuu, 0.0)
            prodpair(cuu, c3(ut)[:, :, 0:62], c3(ut)[:, :, 2:64], 31)
            cwu = main.tile([128, 128], f32)
            nc.gpsimd.memset(cwu, 0.0)
            prodpair(cwu, c3(wt)[:, :, 0:62], c3(ut)[:, :, 2:64], 31, nc.gpsimd, nc.gpsimd)
            cuw = main.tile([128, 128], f32)
            nc.gpsimd.memset(cuw, 0.0)
            prodpair(cuw, c3(ut)[:, :, 0:62], c3(wt)[:, :, 2:64], 31, nc.gpsimd, nc.gpsimd)

            # ---- PCR tiles: [128, 16+128+16]; active cols PAD..PAD+128.
            # System: a_i x_i = b_i + f_{i-s} x_{i-s} + f_i x_{i+s}, f = -e.
            # f is zero at each chunk's last entries, so shifted reads that
            # cross chunk boundaries are multiplied by zero coefficients.
            W, TW = 128, 160

            def A_(t, off=0, w=W):
                return t[:, PAD + off : PAD + off + w]

            apcr = main.tile([128, TW], f32)
            nc.gpsimd.memset(apcr, 1.0)
            ft = [main.tile([128, TW], f32, tag=f"ft{i}", name=f"ft{i}") for i in range(2)]
            bt = [main.tile([128, TW], f32, tag=f"bt{i}", name=f"bt{i}") for i in range(2)]
            for tl_ in (*ft, *bt):
                nc.gpsimd.memset(tl_, 0.0)
            kkp = main.tile([128, TW], f32)
            nc.gpsimd.memset(kkp, 0.0)
            x1p = main.tile([128, TW], f32)
            nc.gpsimd.memset(x1p, 0.0)
            x0p = main.tile([128, TW], f32)
            nc.gpsimd.memset(x0p, 0.0)

            nc.vector.tensor_mul(A_(apcr), jnvP, dcc)
            nc.vector.tensor_mul(A_(ft[0]), fcoP, cuu)
            # b0/2 = pair-sum of u~*w~ written straight into bt[0]; the
            # missing factors fold into the x1e/x0e expansion scales (4.0)
            prodpair(A_(bt[0]), ut, wt, 32, nc.gpsimd, nc.gpsimd)
            cdt = scr.tile([128, 128], f32, tag="cdt", name="cdt")
            nc.vector.tensor_sub(cdt, cwu, cuw)
            nc.vector.tensor_mul(A_(kkp), ecoP, cdt)

            rt_ = main.tile([128, TW], f32)
            # tl[lev][:, 0, j] = t at i = j-s ; tl[lev][:, 1, j] = q at i = j-s
            tl = [main.tile([128, 2, 144], f32, tag=f"tl{i}", name=f"tl{i}") for i in range(5)]
            shifts = [1, 2, 4, 8, 16]

            def P2(tile_, off, ostride, w):
                # paired read: two [128, w] windows at col offset PAD+off and
                # PAD+off+ostride (ostride=0 broadcasts the same window twice)
                b = tile_
                return bass.AP(
                    tensor=b.tensor,
                    offset=b.offset + PAD + off,
                    ap=[b.ap[0], [ostride, 2], [1, w]],
                )

            def pcr_level(lev, cur):
                # t/q pair kept for the rhs-only replay; the a-path engine
                # alternates by level parity so DVE keeps headroom for the
                # MLP's vector links, and the b-path rides the other engine.
                s = shifts[lev]
                W2S = W + s
                ea = nc.vector
                eb = nc.gpsimd
                fC, fN = ft[cur], ft[1 - cur]
                bC, bN = bt[cur], bt[1 - cur]
                nc.vector.reciprocal_approx_fast(
                    A_(rt_, -s, W + 2 * s), A_(apcr, -s, W + 2 * s)
                )
                tq = tl[lev]
                ea.tensor_mul(
                    tq[:, :, 0:W2S], P2(fC, -s, 0, W2S), P2(rt_, -s, s, W2S)
                )
                uw = scr.tile([128, 2, 144], f32, tag="uw", name="uw")
                ea.tensor_mul(
                    uw[:, :, 0:W2S], tq[:, :, 0:W2S], P2(fC, -s, 0, W2S)
                )
                ea.tensor_sub(A_(apcr), A_(apcr), uw[:, 0, 0:W])
                ea.tensor_sub(A_(apcr), A_(apcr), uw[:, 1, s : s + W])
                ea.tensor_mul(A_(fN), tq[:, 1, s : s + W], A_(fC, s))
                tm = scr.tile([128, 2, 144], f32, tag="tm", name="tm")
                eb.tensor_mul(
                    tm[:, :, 0:W2S], tq[:, :, 0:W2S], P2(bC, -s, s, W2S)
                )
                eb.tensor_add(A_(bN), A_(bC), tm[:, 0, 0:W])
                eb.tensor_add(A_(bN), A_(bN), tm[:, 1, s : s + W])

            pcr_level(0, 0)
            pcr_level(1, 1)

            # ================= MLP layer 1 =================
            h1 = main.tile([128, 2, BSL], f32r)
            e1 = main.tile([128, 2, BSL], f32)
            ps1 = psmm.tile([128, 2, BSL], f32, tag="mm", name="ps1")
            for m in range(2):
                nc.tensor.matmul(
                    ps1[:, m, :], wgb[0:1, m * 128 : (m + 1) * 128], ones,
                    start=True, stop=False,
                )
                for k in range(2):
                    nc.tensor.matmul(
                        ps1[:, m, :],
                        w1sb[:, k, m * 128 : (m + 1) * 128],
                        h0[:, k, :],
                        start=False,
                        stop=(k == 1),
                    )
            act_h(ps1, h1)

            pcr_level(2, 0)

            # ================= MLP layer 2 + backward (negated chain) ======
            dp2 = main.tile([128, 2, BSL], f32r)
            ps2 = psmm.tile([128, 2, BSL], f32, tag="mm", name="ps2")
            for m in range(2):
                nc.tensor.matmul(
                    ps2[:, m, :], wgb[0:1, 256 + m * 128 : 256 + (m + 1) * 128],
                    ones, start=True, stop=False,
                )
                for k in range(2):
                    nc.tensor.matmul(
                        ps2[:, m, :],
                        w2sb[:, k, m * 128 : (m + 1) * 128],
                        h1[:, k, :],
                        start=False,
                        stop=(k == 1),
                    )
            # w3 is folded into W2T host-side, so dp2 = e2 - 1 = -sigma2
            h2m = scr.tile([128, 2, BSL], f32, tag="h2m", name="h2m")
            e2m = scr.tile([128, 2, BSL], f32, tag="e2m", name="e2m")
            act_h(ps2, h2m)
            act_e(h2m, e2m)
            nc.scalar.activation(dp2, e2m, AF.Copy, bias=-1.0)

            act_e(h1, e1)
            act_e(h0, e0)

            dp1 = main.tile([128, 2, BSL], f32r)
            ps3 = psmm.tile([128, 2, BSL], f32, tag="mm", name="ps3")
            for m in range(2):
                for k in range(2):
                    nc.tensor.matmul(
                        ps3[:, m, :],
                        w2tsb[:, k, m * 128 : (m + 1) * 128],
                        dp2[:, k, :],
                        start=(k == 0),
                        stop=(k == 1),
                    )
            qb1 = scr.tile([128, 2, BSL], f32, tag="qb", name="qb1")
            nc.vector.tensor_mul(qb1, ps3, e1)
            nc.vector.tensor_sub(dp1, ps3, qb1)
            dp0 = main.tile([128, 2, BSL], f32r)
            ps4 = psmm.tile([128, 2, BSL], f32, tag="mm", name="ps4")
            for m in range(2):
                for k in range(2):
                    nc.tensor.matmul(
                        ps4[:, m, :],
                        w1tsb[:, k, m * 128 : (m + 1) * 128],
                        dp1[:, k, :],
                        start=(k == 0),
                        stop=(k == 1),
                    )
            qb0 = scr.tile([128, 2, BSL], f32, tag="qb", name="qb0")
            nc.vector.tensor_mul(qb0, ps4, e0)
            nc.vector.tensor_sub(dp0, ps4, qb0)

            pcr_level(3, 1)
            # -g batch-major DIRECTLY: per batch chunk c, out[b, rfeat] =
            # dp0[:, :, c-cols]^T @ W0T  (no transposes / copies needed; the
            # column ordering of dp0 matches the zt batch permutation)
            gbm3 = psmm.tile([128, C4, 64], f32, tag="mm", name="gbm3")
            for c in range(C4):
                for k in range(2):
                    nc.tensor.matmul(
                        gbm3[:, c, :],
                        dp0[:, k, c * 128 : (c + 1) * 128],
                        w0tsb[:, k, :],
                        start=(k == 0),
                        stop=(k == 1),
                    )

            pcr_level(4, 0)
            nc.vector.reciprocal_approx_fast(A_(rt_), A_(apcr))
            nc.vector.tensor_mul(A_(x1p), A_(bt[1]), A_(rt_))

            # ---- x1-dependent outputs (out_r) ----
            outt = main.tile([128, C4, ZD], f32)
            outv = out.ap().rearrange("(p j) f -> p j f", j=4)
            x1e = main.tile([128, 256], f32)
            xv1 = x1e.rearrange("p (c i d) -> p c i d", d=2, i=32)
            x1a3 = A_(x1p).rearrange("p (c i) -> p c i", i=32)
            nc.scalar.activation(xv1[:, :, :, 0], x1a3, AF.Copy, scale=4.0)
            nc.scalar.activation(xv1[:, :, :, 1], x1a3, AF.Copy, scale=4.0)

            def drx(dst3, src3, dve_tail=False):
                # dst = Dr-combine(src): node0: s0+s1; mid: s_{i+1}-s_i; last: -s
                nc.vector.tensor_add(
                    dst3[:, :, 0:2], src3[:, :, 2:4], src3[:, :, 0:2]
                )
                nc.vector.tensor_sub(
                    dst3[:, :, 2:62], src3[:, :, 4:64], src3[:, :, 2:62]
                )
                if dve_tail:
                    nc.vector.tensor_scalar_mul(
                        dst3[:, :, 62:64], src3[:, :, 62:64], -1.0
                    )
                else:
                    nc.scalar.activation(
                        dst3[:, :, 62:64], src3[:, :, 62:64], AF.Copy, scale=-1.0
                    )

            A1 = main.tile([128, 256], f32)        # x1 * u (per feature)
            nc.vector.tensor_mul(A1, x1e, ut)
            drA = main.tile([128, 256], f32)
            drx(c3(drA), c3(A1))
            sD = scr.tile([128, 256], f32, tag="sD", name="sD")
            nc.vector.tensor_mul(sD, drA, invmP)
            nc.vector.tensor_sub(outt[:, :, 0:64], c3(vt), c3(sD))
            nc.sync.dma_start(out=outv[:, :, 0:64], in_=outt[:, :, 0:64])
            bs_ = scr.tile([128, 256], f32, tag="bs_", name="bs_")
            nc.vector.tensor_mul(bs_, x1e, wt)

            # ---- b1 pieces that need g (gt/gd/pugd carry -g signs) ----
            gt = main.tile([128, 256], f32)        # -Minv g
            nc.vector.tensor_mul(c3(gt), gbm3, invmP3)
            gd = main.tile([128, 256], f32)        # -chain-diff of Minv g
            nc.vector.tensor_sub(
                c3(gd)[:, :, 2:64], c3(gt)[:, :, 0:62], c3(gt)[:, :, 2:64]
            )
            nc.vector.tensor_copy(c3(gd)[:, :, 0:2], c3(gt)[:, :, 0:2])
            pugd = main.tile([128, 128], f32)      # = -(u~ . Gd)
            prodpair(pugd, ut, gd, 32)
            b1t = main.tile([128, 128], f32)       # (b1 = 2*b1t)
            nc.vector.tensor_add(b1t, pww, pugd)

            # ---- rhs2 = K x1 - b1; solve G x0 = rhs2 by replaying PCR ----
            t1 = scr.tile([128, 128], f32, tag="t1", name="t1")
            nc.vector.tensor_mul(t1, A_(kkp), A_(x1p, 1))
            t2k = scr.tile([128, 128], f32, tag="t2k", name="t2k")
            nc.vector.tensor_mul(t2k, A_(kkp, -1), A_(x1p, -1))
            ttk = scr.tile([128, 128], f32, tag="ttk", name="ttk")
            nc.vector.tensor_sub(ttk, t1, t2k)
            nc.vector.scalar_tensor_tensor(
                A_(bt[0]), b1t, -1.0, ttk, op0=OP.mult, op1=OP.add
            )
            cur = 0
            for lev, s in enumerate(shifts):
                W2S = W + s
                bC, bN = bt[cur], bt[1 - cur]
                tm = scr.tile([128, 2, 144], f32, tag="tm", name="tm")
                nc.vector.tensor_mul(
                    tm[:, :, 0:W2S], tl[lev][:, :, 0:W2S], P2(bC, -s, s, W2S)
                )
                nc.vector.tensor_add(A_(bN), A_(bC), tm[:, 0, 0:W])
                nc.vector.tensor_add(A_(bN), A_(bN), tm[:, 1, s : s + W])
                cur = 1 - cur
            nc.vector.tensor_mul(A_(x0p), A_(bt[1]), A_(rt_))

            # ---- x0-dependent outputs (out_p) ----
            x0e = main.tile([128, 256], f32)
            xv0 = x0e.rearrange("p (c i d) -> p c i d", d=2, i=32)
            x0a3 = A_(x0p).rearrange("p (c i) -> p c i", i=32)
            nc.vector.tensor_scalar_mul(xv0[:, :, :, 0], x0a3, 4.0)
            nc.vector.tensor_scalar_mul(xv0[:, :, :, 1], x0a3, 4.0)
            Bt1 = main.tile([128, 256], f32)       # x0*u + x1*w
            nc.vector.tensor_mul(Bt1, x0e, ut)
            nc.vector.tensor_add(Bt1, Bt1, bs_)
            drB = main.tile([128, 256], f32)
            drx(c3(drB), c3(Bt1), dve_tail=True)
            nc.vector.tensor_add(outt[:, :, 64:128], c3(drB), gbm3)
            nc.sync.dma_start(out=outv[:, :, 64:128], in_=outt[:, :, 64:128])

    nc.compile()
    return nc


def host_inputs(inputs):
    """Host-side prep: per-core input maps (weights replicated, z sharded)."""
    f = lambda x: np.ascontiguousarray(np.asarray(x, np.float32))
    z = f(inputs["z"])
    W0, W1, W2, W3 = f(inputs["W0"]), f(inputs["W1"]), f(inputs["W2"]), f(inputs["W3"])

    wpk = np.zeros((128, WTOT), np.float32)
    wpk[0:64, OW0 : OW0 + 256] = W0
    wpk[64, OW0 : OW0 + 256] = f(inputs["b0"])          # ones-row bias
    W2W = np.ascontiguousarray(W2 * W3[:, 0][None, :])  # fold w3 into W2^T
    for k in range(2):
        sl = slice(k * 128, (k + 1) * 128)
        wpk[:, OW1 + 256 * k : OW1 + 256 * (k + 1)] = W1[sl]
        wpk[:, OW2 + 256 * k : OW2 + 256 * (k + 1)] = W2[sl]
        wpk[:, OW1T + 256 * k : OW1T + 256 * (k + 1)] = W1.T[sl]
        wpk[:, OW2T + 256 * k : OW2T + 256 * (k + 1)] = W2W.T[sl]
        wpk[:, OW0T + 64 * k : OW0T + 64 * (k + 1)] = W0.T[sl]
    wpk[0, OBR : OBR + 256] = f(inputs["b1"])
    wpk[0, OBR + 256 : OBR + 512] = f(inputs["b2"])
    wpk[0, OON : OON + BSL] = 1.0

    inv = np.exp(-f(inputs["m_params"])[:, 0])
    invm64 = np.repeat(inv, 2)
    jnv = np.empty(32, np.float32)
    jnv[0] = inv[0]
    jnv[1:] = inv[:-1] + inv[1:]
    eco = (-4.0 * inv).astype(np.float32)   # sigma_c * 4 * inv_c, c>=1
    eco[0] = 4.0 * inv[0]
    eco[31] = 0.0
    row = np.zeros(CTOT, np.float32)
    row[OIV : OIV + 256] = np.tile(invm64, 4)
    row[OJN : OJN + 128] = np.tile(4.0 * jnv, 4)
    row[OFC : OFC + 128] = np.tile(-eco, 4)   # f = -e
    row[OEC : OEC + 128] = np.tile(eco, 4)
    cpk = np.broadcast_to(row, (128, CTOT)).copy()
    for off, b in ((OB0, inputs["b0"]), (OB1, inputs["b1"]), (OB2, inputs["b2"])):
        cpk[:, off : off + 2] = f(b).reshape(2, 128).T
    cpk[:, OW3 : OW3 + 2] = W3[:, 0].reshape(2, 128).T
    cpk = np.ascontiguousarray(cpk)

    shared = {"wpk": np.ascontiguousarray(wpk), "cpk": cpk}
    return [
        {**shared, "z": np.ascontiguousarray(z[i * BSL : (i + 1) * BSL])}
        for i in range(NCORES)
    ]


TRACE = False       # set by dev harnesses to capture an NTFF profile
TMPDIR = None       # set by dev harnesses to keep the trace artifacts
LAST_RESULT = None  # BassKernelResults of the most recent run


def kernel(**inputs) -> np.ndarray:
    global LAST_RESULT
    from concourse.bass_utils import run_bass_kernel_spmd

    nc = build_program()
    in_maps = host_inputs(inputs)
    res = run_bass_kernel_spmd(
        nc, in_maps, list(range(NCORES)), trace=TRACE, tmpdir=TMPDIR
    )
    LAST_RESULT = res
    return np.concatenate([res.results[i]["out"] for i in range(NCORES)], axis=0)
